# revision 6
# baseline (speedup 1.0000x reference)
"""GATv2 attention layer (B=2, T=1024, C_IN=128, D=64) on 8 trn2 NeuronCores.

Sharding: flatten (B, T) destination rows -> 2048 rows, 256 per core.
Host packs inputs per core: qT2 = [q^T; q^T] fp16 (q = feat@W1^T), kpair
fp32 columns [k[2p]; k[2p+1]] (k = feat@W2^T), feat in 128-row blocks with
a ones column (rowsum trick), transposed adj tiles, the a-weight slot
matrix A32s, and an identity for PE transposes.

Per-core algorithm (i = destination row, j = source node, d = head dim 64):
  scores[i, j] = sum_d a[d] * relu(q[j, d] + k[i, d])
E2 tile per row-pair: relu(qT2 + kpair[:, p]) on DVE (396ns, 4x mode) or
ACT (1061ns) -- split ~47/17 per 64-pair i-tile to balance both engines.
Scores via PE matmuls with lhsT = A32s slot q, 16 pairs accumulating into
a 32-row psum band; the two 512-col halves of a pair are emitted skewed
(s0 of pair idx, then s1 of pair idx-1) so consecutive matmuls hit
different PE column bands -- the PE streams up to 4 cols/cycle across 4
concurrent bands instead of serializing at 1 col/cycle.
Softmax: exp without row-max stabilizer (scores bounded); adj mask folded
into the post-transpose PSUM eviction (attT = pst * adjT).  The whole
i-tile tail (exp/transpose/mask/att-matmul/norm) is software-pipelined
into the NEXT i-tile's E2 stream so no engine drains at the boundary.
Final: out[i, :] = (att_unnorm @ [feat|1]) / rowsum.
"""
import sys

sys.path.insert(0, "/opt/trn_rl_repo")

from contextlib import ExitStack

import numpy as np

import concourse.bass as bass  # noqa: F401
import concourse.tile as tile
from concourse import bacc, mybir
from concourse.bass_utils import run_bass_kernel_spmd

B, T, C_IN, D = 2, 1024, 128, 64
N_CORES = 8
ROWS = (B * T) // N_CORES  # 256 destination rows per core
CPB = N_CORES // B  # cores per batch
NT = T // 128  # token tiles
NIT = ROWS // 128  # i-tiles per core
NPAIR = 64  # row pairs per i-tile
NSLOT = 16  # pair slots per 32-row psum band

FP32 = mybir.dt.float32
FP16 = mybir.dt.float16
OP = mybir.AluOpType
AF = mybir.ActivationFunctionType


def _on_scalar(idx):
    # 17 of 64 E2 tiles per i-tile on the ACT engine
    return idx % 4 == 2 or idx == 33


def _emit(ctx, tc, nc, hot_in, cold_in, adjT, out):
    singles = ctx.enter_context(tc.tile_pool(name="singles", bufs=1))
    hot = singles.tile([128, T + ROWS // 2 + NSLOT * 32], FP16)
    cold = singles.tile([128, 128 + NT * (C_IN + 1)], FP16)
    dum = singles.tile([128, 2], FP16)
    qT2 = hot[:, 0:T]
    kpair16 = hot[:, T : T + ROWS // 2]
    A32s = hot[:, T + ROWS // 2 :]
    kpair = singles.tile([128, ROWS // 2], FP32)
    ident16 = cold[:, 0:128]
    feat16 = cold[:, 128:]

    # DMA triggers cost ~650ns serial per issuing sequencer: one hot DMA
    # first on sync, cold/adjT on the otherwise idle gpsimd sequencer.
    nc.sync.dma_start(hot[:], hot_in[:, :])
    nc.gpsimd.dma_start(cold[:], cold_in[:, :])
    adj_sb = []
    adjpool = ctx.enter_context(tc.tile_pool(name="adjp", bufs=NIT))
    for it in range(NIT):
        at = adjpool.tile([128, T], FP16, tag="adjT", name=f"adjT_{it}")
        nc.gpsimd.dma_start(at[:], adjT[:, it * T : (it + 1) * T])
        adj_sb.append(at)

    # trigger the ACT table load (Exp) immediately, no data deps
    nc.scalar.memzero(dum[:])
    nc.scalar.activation(dum[:], dum[:], AF.Exp)
    # tensor_scalar needs an fp32 scalar: upconvert kpair once on ACT
    nc.scalar.copy(kpair[:], kpair16)

    e2pool = ctx.enter_context(tc.tile_pool(name="e2", bufs=8))
    softpool = ctx.enter_context(tc.tile_pool(name="soft", bufs=2))
    smallpool = ctx.enter_context(tc.tile_pool(name="small", bufs=2))
    attTpool = ctx.enter_context(tc.tile_pool(name="attT", bufs=2))
    outpool = ctx.enter_context(tc.tile_pool(name="outp", bufs=2))
    ps_scores = ctx.enter_context(tc.tile_pool(name="ps_s", bufs=2, space="PSUM"))
    ps_tr = ctx.enter_context(tc.tile_pool(name="ps_tr", bufs=2, space="PSUM"))
    ps_out = ctx.enter_context(tc.tile_pool(name="ps_o", bufs=2, space="PSUM"))

    W = C_IN + 1
    state = {}  # tail state of the previous i-tile

    def tail_exp(st, lo, step):
        nc.scalar.activation(
            st["pexp"][:, lo : lo + step], st["s"][:, lo : lo + step], AF.Exp
        )

    def tail_transpose(st, t):
        nc.tensor.transpose(
            st["pst"][:, t * 128 : (t + 1) * 128],
            st["pexp"][:, t * 128 : (t + 1) * 128],
            ident16,
        )

    def tail_mask(st, lo, step):
        nc.vector.tensor_tensor(
            st["attT"][:, lo : lo + step],
            st["pst"][:, lo : lo + step],
            adj_sb[st["it"]][:, lo : lo + step],
            OP.mult,
        )

    def tail_attmm(st, t):
        nc.tensor.matmul(
            st["po"][:],
            st["attT"][:, t * 128 : (t + 1) * 128],
            feat16[:, t * W : (t + 1) * W],
            start=(t == 0),
            stop=(t == NT - 1),
            skip_group_check=True,
        )

    def tail_norm_and_out(st):
        inv = smallpool.tile([128, 1], FP32, tag="inv")
        nc.vector.reciprocal(inv[:], st["po"][:, C_IN : C_IN + 1])
        out_sb = outpool.tile([128, C_IN], FP32, tag="out")
        nc.scalar.mul(out_sb[:], st["po"][:, 0:C_IN], inv[:])
        it = st["it"]
        nc.gpsimd.dma_start(out[it * 128 : (it + 1) * 128, :], out_sb[:])

    for it in range(NIT):
        s = ps_scores.tile([128, T], FP32, tag="s")  # 2 banks; halves by matmul
        prev = state if it > 0 else None
        pend = None  # (e2, lhsT, g, first, last) awaiting its 512:T half
        for idx in range(NPAIR):
            q, g = divmod(idx, 4)
            p = NSLOT * g + q
            P = it * NPAIR + p
            e2 = e2pool.tile([128, T], FP16, tag="e2")
            kcol = kpair[:, P : P + 1]
            if _on_scalar(idx):
                nc.scalar.activation(e2[:], qT2, AF.Relu, bias=kcol)
            else:
                nc.vector.tensor_scalar(e2[:], qT2, kcol, 0.0, OP.add, OP.max)
            lhsT = A32s[:, 32 * q : 32 * q + 32]
            first, last = q == 0, q == NSLOT - 1
            nc.tensor.matmul(
                s[32 * g : 32 * g + 32, 0:512],
                lhsT,
                e2[:, 0:512],
                start=first,
                stop=last,
                tile_position=(0, 32 * g),
                skip_group_check=True,
            )
            if pend is not None:
                pe2, plhsT, pg, pfirst, plast = pend
                nc.tensor.matmul(
                    s[32 * pg : 32 * pg + 32, 512:T],
                    plhsT,
                    pe2[:, 512:T],
                    start=pfirst,
                    stop=plast,
                    tile_position=(0, 32 * pg),
                    skip_group_check=True,
                )
            pend = (e2, lhsT, g, first, last)

            # software-pipelined tail of the previous i-tile
            if prev is not None:
                if idx == 4:
                    tail_exp(prev, 0, T)
                elif 6 <= idx <= 13:
                    tail_transpose(prev, idx - 6)
                elif idx == 15:
                    tail_mask(prev, 0, T)
                elif 17 <= idx <= 24:
                    tail_attmm(prev, idx - 17)
                elif idx == 26:
                    tail_norm_and_out(prev)

        pe2, plhsT, pg, pfirst, plast = pend
        nc.tensor.matmul(
            s[32 * pg : 32 * pg + 32, 512:T],
            plhsT,
            pe2[:, 512:T],
            start=pfirst,
            stop=plast,
            tile_position=(0, 32 * pg),
            skip_group_check=True,
        )

        state = {
            "it": it,
            "s": s,
            "pexp": softpool.tile([128, T], FP16, tag="pexp", name=f"pexp_{it}"),
            "pst": ps_tr.tile([128, T], FP16, tag="tr", name=f"pst_{it}"),
            "attT": attTpool.tile([128, T], FP16, tag="attT", name=f"attT_{it}"),
            "po": ps_out.tile([128, W], FP32, tag="o", name=f"po_{it}"),
        }

    # final i-tile's tail, pipelined in j-quarters to shorten the exposed chain
    st = state
    for hh in range(4):
        lo = hh * 256
        tail_exp(st, lo, 256)
        for t in range(lo // 128, (lo + 256) // 128):
            tail_transpose(st, t)
        tail_mask(st, lo, 256)
        for t in range(lo // 128, (lo + 256) // 128):
            tail_attmm(st, t)
    tail_norm_and_out(st)


_PROGRAM = None


def build_program():
    global _PROGRAM
    if _PROGRAM is not None:
        return _PROGRAM
    nc = bacc.Bacc("TRN2", target_bir_lowering=False, debug=False, num_devices=N_CORES)
    hot_in = nc.dram_tensor("hot", [128, T + ROWS // 2 + NSLOT * 32], FP16, kind="ExternalInput")
    cold_in = nc.dram_tensor("cold", [128, 128 + NT * (C_IN + 1)], FP16, kind="ExternalInput")
    adjT = nc.dram_tensor("adjT", [128, NIT * T], FP16, kind="ExternalInput")
    out = nc.dram_tensor("out", [ROWS, C_IN], FP32, kind="ExternalOutput")
    with tile.TileContext(nc) as tc:
        with ExitStack() as ctx:
            _emit(ctx, tc, nc, hot_in, cold_in, adjT, out)
    nc.compile()
    _PROGRAM = nc
    return nc


def make_a32(a):
    a32 = np.zeros((128, NSLOT * 32), dtype=np.float16)
    for q in range(NSLOT):
        a32[0:64, 32 * q + 2 * q] = a
        a32[64:128, 32 * q + 2 * q + 1] = a
    return a32


def make_in_maps(feat, adj, W1, W2, a):
    feat = np.ascontiguousarray(feat, dtype=np.float32)
    adj = np.ascontiguousarray(adj, dtype=np.float32)
    W1 = np.asarray(W1, dtype=np.float32)
    W2 = np.asarray(W2, dtype=np.float32)
    a32 = make_a32(np.asarray(a, dtype=np.float32))
    ident = np.eye(128, dtype=np.float16)
    in_maps = []
    for b in range(B):
        feat16 = feat[b].astype(np.float16)  # [T, C_IN]
        q = (feat16.astype(np.float32) @ W1.T).astype(np.float16)  # [T, D]
        k = feat16.astype(np.float32) @ W2.T  # [T, D] fp32
        qT = np.ascontiguousarray(q.T)  # [D, T]
        qT2 = np.ascontiguousarray(np.concatenate([qT, qT], axis=0))  # [128, T]
        fb = feat16.reshape(NT, 128, C_IN).transpose(1, 0, 2)  # [128, NT, C_IN]
        fblk = np.concatenate(
            [fb, np.ones((128, NT, 1), dtype=np.float16)], axis=2
        ).reshape(128, NT * (C_IN + 1))
        fblk = np.ascontiguousarray(fblk)
        for cc in range(CPB):
            r0 = cc * ROWS
            krows = k[r0 : r0 + ROWS].astype(np.float32)  # [256, 64]
            kp = krows.reshape(ROWS // 2, 2, D)  # [p, two, d]
            kpair = np.ascontiguousarray(
                kp.transpose(1, 2, 0).reshape(128, ROWS // 2).astype(np.float32)
            )  # [[k2p d]; [k2p+1 d]] stacked -> [128, 128]
            arows = adj[b, r0 : r0 + ROWS].astype(np.float16)  # [256, 1024]
            ats = []
            for it in range(NIT):
                chunk = arows[it * 128 : (it + 1) * 128]  # [128 i, 1024 j]
                x = chunk.reshape(128, NT, 128)  # [ii, t, p]
                ats.append(x.transpose(2, 1, 0).reshape(128, T))  # [p, (t, ii)]
            adjT = np.ascontiguousarray(np.concatenate(ats, axis=1))
            hot = np.ascontiguousarray(
                np.concatenate([qT2, kpair.astype(np.float16), a32], axis=1)
            )
            cold = np.ascontiguousarray(np.concatenate([ident, fblk], axis=1))
            in_maps.append({"hot": hot, "cold": cold, "adjT": adjT})
    return in_maps


def run(feat, adj, W1, W2, a, trace=False):
    nc = build_program()
    in_maps = make_in_maps(feat, adj, W1, W2, a)
    last_err = None
    for attempt in range(3):
        try:
            res = run_bass_kernel_spmd(
                nc, in_maps, core_ids=list(range(N_CORES)), trace=trace
            )
            outs = [np.asarray(res.results[c]["out"]) for c in range(N_CORES)]
            break
        except Exception as e:  # transient NRT device errors recover on retry
            last_err = e
            import time

            time.sleep(5)
    else:
        raise last_err
    full = np.concatenate(outs, axis=0).reshape(B, T, C_IN).astype(np.float32)
    return full, res


def kernel(feat, adj, W1, W2, a):
    full, _ = run(feat, adj, W1, W2, a)
    return full


# revision 7
# speedup vs baseline: 1.1347x; 1.1347x over previous
"""GATv2 attention layer (B=2, T=1024, C_IN=128, D=64) on 8 trn2 NeuronCores.

Sharding: flatten (B, T) destination rows -> 2048 rows, 256 per core.
Host packs inputs per core: qT2 = [q^T; q^T] fp16 (q = feat@W1^T), kpair
fp32 columns [k[2p]; k[2p+1]] (k = feat@W2^T), feat in 128-row blocks with
a ones column (rowsum trick), transposed adj tiles, the a-weight slot
matrix A32s, and an identity for PE transposes.

Per-core algorithm (i = destination row, j = source node, d = head dim 64):
  scores[i, j] = sum_d a[d] * relu(q[j, d] + k[i, d])
E2 tile per row-pair: relu(qT2 + kpair[:, p]) on DVE (396ns, 4x mode) or
ACT (1061ns) -- split ~47/17 per 64-pair i-tile to balance both engines.
Scores via PE matmuls with lhsT = A32s slot q, 16 pairs accumulating into
a 32-row psum band; the two 512-col halves of a pair are emitted skewed
(s0 of pair idx, then s1 of pair idx-1) so consecutive matmuls hit
different PE column bands -- the PE streams up to 4 cols/cycle across 4
concurrent bands instead of serializing at 1 col/cycle.
Softmax: exp without row-max stabilizer (scores bounded); adj mask folded
into the post-transpose PSUM eviction (attT = pst * adjT).  The whole
i-tile tail (exp/transpose/mask/att-matmul/norm) is software-pipelined
into the NEXT i-tile's E2 stream so no engine drains at the boundary.
Final: out[i, :] = (att_unnorm @ [feat|1]) / rowsum.
"""
import sys

sys.path.insert(0, "/opt/trn_rl_repo")

from contextlib import ExitStack

import numpy as np

import concourse.bass as bass  # noqa: F401
import concourse.tile as tile
from concourse import bacc, mybir
from concourse.bass_utils import run_bass_kernel_spmd

B, T, C_IN, D = 2, 1024, 128, 64
N_CORES = 8
ROWS = (B * T) // N_CORES  # 256 destination rows per core
CPB = N_CORES // B  # cores per batch
NT = T // 128  # token tiles
NIT = ROWS // 128  # i-tiles per core
NPAIR = 64  # row pairs per i-tile
NSLOT = 16  # pair slots per 32-row psum band

FP32 = mybir.dt.float32
FP16 = mybir.dt.float16
OP = mybir.AluOpType
AF = mybir.ActivationFunctionType


def _on_scalar(idx):
    # 17 of 64 E2 tiles per i-tile on the ACT engine
    return idx % 4 == 2 or idx == 33


def _emit(ctx, tc, nc, hot_in, cold_in, adjT, out):
    singles = ctx.enter_context(tc.tile_pool(name="singles", bufs=1))
    hot = singles.tile([128, 2048], FP16)  # power-of-two row pitch
    cold = singles.tile([128, 128 + NT * (C_IN + 1)], FP16)
    dum = singles.tile([128, 2], FP16)
    qT2 = hot[:, 0:T]
    kpair16 = hot[:, T : T + ROWS // 2]
    A32s = hot[:, T + ROWS // 2 :]
    kpair = singles.tile([128, ROWS // 2], FP32)
    ident16 = cold[:, 0:128]
    feat16 = cold[:, 128:]

    # DMA triggers cost ~650ns serial per issuing sequencer: one hot DMA
    # first on sync, cold/adjT on the otherwise idle gpsimd sequencer.
    nc.sync.dma_start(hot[:], hot_in[:, :])
    nc.gpsimd.dma_start(cold[:], cold_in[:, :])
    adj_sb = []
    adjpool = ctx.enter_context(tc.tile_pool(name="adjp", bufs=NIT))
    for it in range(NIT):
        at = adjpool.tile([128, T], FP16, tag="adjT", name=f"adjT_{it}")
        nc.gpsimd.dma_start(at[:], adjT[:, it * T : (it + 1) * T])
        adj_sb.append(at)

    # trigger the ACT table load (Exp) immediately, no data deps
    nc.scalar.memzero(dum[:])
    nc.scalar.activation(dum[:], dum[:], AF.Exp)
    # tensor_scalar needs an fp32 scalar: upconvert kpair once on ACT
    nc.scalar.copy(kpair[:], kpair16)

    e2pool = ctx.enter_context(tc.tile_pool(name="e2", bufs=8))
    softpool = ctx.enter_context(tc.tile_pool(name="soft", bufs=2))
    smallpool = ctx.enter_context(tc.tile_pool(name="small", bufs=2))
    attTpool = ctx.enter_context(tc.tile_pool(name="attT", bufs=2))
    outpool = ctx.enter_context(tc.tile_pool(name="outp", bufs=2))
    ps_scores = ctx.enter_context(tc.tile_pool(name="ps_s", bufs=2, space="PSUM"))
    ps_tr = ctx.enter_context(tc.tile_pool(name="ps_tr", bufs=2, space="PSUM"))
    ps_out = ctx.enter_context(tc.tile_pool(name="ps_o", bufs=2, space="PSUM"))

    W = C_IN + 1
    state = {}  # tail state of the previous i-tile

    def tail_exp(st, lo, step):
        nc.scalar.activation(
            st["pexp"][:, lo : lo + step], st["s"][:, lo : lo + step], AF.Exp
        )

    def tail_transpose(st, t):
        nc.tensor.transpose(
            st["pst"][:, t * 128 : (t + 1) * 128],
            st["pexp"][:, t * 128 : (t + 1) * 128],
            ident16,
        )

    def tail_mask(st, lo, step):
        nc.vector.tensor_tensor(
            st["attT"][:, lo : lo + step],
            st["pst"][:, lo : lo + step],
            adj_sb[st["it"]][:, lo : lo + step],
            OP.mult,
        )

    def tail_attmm(st, t):
        nc.tensor.matmul(
            st["po"][:],
            st["attT"][:, t * 128 : (t + 1) * 128],
            feat16[:, t * W : (t + 1) * W],
            start=(t == 0),
            stop=(t == NT - 1),
            skip_group_check=True,
        )

    def tail_norm_and_out(st):
        inv = smallpool.tile([128, 1], FP32, tag="inv")
        nc.vector.reciprocal(inv[:], st["po"][:, C_IN : C_IN + 1])
        out_sb = outpool.tile([128, C_IN], FP32, tag="out")
        nc.scalar.mul(out_sb[:], st["po"][:, 0:C_IN], inv[:])
        it = st["it"]
        nc.gpsimd.dma_start(out[it * 128 : (it + 1) * 128, :], out_sb[:])

    for it in range(NIT):
        s = ps_scores.tile([128, T], FP32, tag="s")  # 2 banks; halves by matmul
        prev = state if it > 0 else None
        pend = None  # (e2, lhsT, g, first, last) awaiting its 512:T half
        for idx in range(NPAIR):
            q, g = divmod(idx, 4)
            p = NSLOT * g + q
            P = it * NPAIR + p
            e2 = e2pool.tile([128, T], FP16, tag="e2")
            kcol = kpair[:, P : P + 1]
            if _on_scalar(idx):
                nc.scalar.activation(e2[:], qT2, AF.Relu, bias=kcol)
            else:
                nc.vector.tensor_scalar(e2[:], qT2, kcol, 0.0, OP.add, OP.max)
            lhsT = A32s[:, 32 * q : 32 * q + 32]
            first, last = q == 0, q == NSLOT - 1
            nc.tensor.matmul(
                s[32 * g : 32 * g + 32, 0:512],
                lhsT,
                e2[:, 0:512],
                start=first,
                stop=last,
                tile_position=(0, 32 * g),
                skip_group_check=True,
            )
            if pend is not None:
                pe2, plhsT, pg, pfirst, plast = pend
                nc.tensor.matmul(
                    s[32 * pg : 32 * pg + 32, 512:T],
                    plhsT,
                    pe2[:, 512:T],
                    start=pfirst,
                    stop=plast,
                    tile_position=(0, 32 * pg),
                    skip_group_check=True,
                )
            pend = (e2, lhsT, g, first, last)

            # software-pipelined tail of the previous i-tile
            if prev is not None:
                if idx == 4:
                    tail_exp(prev, 0, T)
                elif 6 <= idx <= 13:
                    tail_transpose(prev, idx - 6)
                elif idx == 15:
                    tail_mask(prev, 0, T)
                elif 17 <= idx <= 24:
                    tail_attmm(prev, idx - 17)
                elif idx == 26:
                    tail_norm_and_out(prev)

        pe2, plhsT, pg, pfirst, plast = pend
        nc.tensor.matmul(
            s[32 * pg : 32 * pg + 32, 512:T],
            plhsT,
            pe2[:, 512:T],
            start=pfirst,
            stop=plast,
            tile_position=(0, 32 * pg),
            skip_group_check=True,
        )

        state = {
            "it": it,
            "s": s,
            "pexp": softpool.tile([128, T], FP16, tag="pexp", name=f"pexp_{it}"),
            "pst": ps_tr.tile([128, T], FP16, tag="tr", name=f"pst_{it}"),
            "attT": attTpool.tile([128, T], FP16, tag="attT", name=f"attT_{it}"),
            "po": ps_out.tile([128, W], FP32, tag="o", name=f"po_{it}"),
        }

    # final i-tile's tail, pipelined in j-quarters to shorten the exposed chain
    st = state
    for hh in range(4):
        lo = hh * 256
        tail_exp(st, lo, 256)
        for t in range(lo // 128, (lo + 256) // 128):
            tail_transpose(st, t)
        tail_mask(st, lo, 256)
        for t in range(lo // 128, (lo + 256) // 128):
            tail_attmm(st, t)
    tail_norm_and_out(st)


_PROGRAM = None


def build_program():
    global _PROGRAM
    if _PROGRAM is not None:
        return _PROGRAM
    nc = bacc.Bacc("TRN2", target_bir_lowering=False, debug=False, num_devices=N_CORES)
    hot_in = nc.dram_tensor("hot", [128, 2048], FP16, kind="ExternalInput")
    cold_in = nc.dram_tensor("cold", [128, 128 + NT * (C_IN + 1)], FP16, kind="ExternalInput")
    adjT = nc.dram_tensor("adjT", [128, NIT * T], FP16, kind="ExternalInput")
    out = nc.dram_tensor("out", [ROWS, C_IN], FP32, kind="ExternalOutput")
    with tile.TileContext(nc) as tc:
        with ExitStack() as ctx:
            _emit(ctx, tc, nc, hot_in, cold_in, adjT, out)
    nc.compile()
    _PROGRAM = nc
    return nc


def make_a32(a):
    a32 = np.zeros((128, NSLOT * 32), dtype=np.float16)
    for q in range(NSLOT):
        a32[0:64, 32 * q + 2 * q] = a
        a32[64:128, 32 * q + 2 * q + 1] = a
    return a32


def make_in_maps(feat, adj, W1, W2, a):
    feat = np.ascontiguousarray(feat, dtype=np.float32)
    adj = np.ascontiguousarray(adj, dtype=np.float32)
    W1 = np.asarray(W1, dtype=np.float32)
    W2 = np.asarray(W2, dtype=np.float32)
    a32 = make_a32(np.asarray(a, dtype=np.float32))
    ident = np.eye(128, dtype=np.float16)
    in_maps = []
    for b in range(B):
        feat16 = feat[b].astype(np.float16)  # [T, C_IN]
        q = (feat16.astype(np.float32) @ W1.T).astype(np.float16)  # [T, D]
        k = feat16.astype(np.float32) @ W2.T  # [T, D] fp32
        qT = np.ascontiguousarray(q.T)  # [D, T]
        qT2 = np.ascontiguousarray(np.concatenate([qT, qT], axis=0))  # [128, T]
        fb = feat16.reshape(NT, 128, C_IN).transpose(1, 0, 2)  # [128, NT, C_IN]
        fblk = np.concatenate(
            [fb, np.ones((128, NT, 1), dtype=np.float16)], axis=2
        ).reshape(128, NT * (C_IN + 1))
        fblk = np.ascontiguousarray(fblk)
        for cc in range(CPB):
            r0 = cc * ROWS
            krows = k[r0 : r0 + ROWS].astype(np.float32)  # [256, 64]
            kp = krows.reshape(ROWS // 2, 2, D)  # [p, two, d]
            kpair = np.ascontiguousarray(
                kp.transpose(1, 2, 0).reshape(128, ROWS // 2).astype(np.float32)
            )  # [[k2p d]; [k2p+1 d]] stacked -> [128, 128]
            arows = adj[b, r0 : r0 + ROWS].astype(np.float16)  # [256, 1024]
            ats = []
            for it in range(NIT):
                chunk = arows[it * 128 : (it + 1) * 128]  # [128 i, 1024 j]
                x = chunk.reshape(128, NT, 128)  # [ii, t, p]
                ats.append(x.transpose(2, 1, 0).reshape(128, T))  # [p, (t, ii)]
            adjT = np.ascontiguousarray(np.concatenate(ats, axis=1))
            hot = np.zeros((128, 2048), dtype=np.float16)
            hot[:, 0:T] = qT2
            hot[:, T : T + ROWS // 2] = kpair.astype(np.float16)
            hot[:, T + ROWS // 2 : T + ROWS // 2 + NSLOT * 32] = a32
            cold = np.ascontiguousarray(np.concatenate([ident, fblk], axis=1))
            in_maps.append({"hot": hot, "cold": cold, "adjT": adjT})
    return in_maps


def run(feat, adj, W1, W2, a, trace=False):
    nc = build_program()
    in_maps = make_in_maps(feat, adj, W1, W2, a)
    last_err = None
    for attempt in range(3):
        try:
            res = run_bass_kernel_spmd(
                nc, in_maps, core_ids=list(range(N_CORES)), trace=trace
            )
            outs = [np.asarray(res.results[c]["out"]) for c in range(N_CORES)]
            break
        except Exception as e:  # transient NRT device errors recover on retry
            last_err = e
            import time

            time.sleep(5)
    else:
        raise last_err
    full = np.concatenate(outs, axis=0).reshape(B, T, C_IN).astype(np.float32)
    return full, res


def kernel(feat, adj, W1, W2, a):
    full, _ = run(feat, adj, W1, W2, a)
    return full


# revision 8
# speedup vs baseline: 1.1486x; 1.0122x over previous
"""GATv2 attention layer (B=2, T=1024, C_IN=128, D=64) on 8 trn2 NeuronCores.

Sharding: flatten (B, T) destination rows -> 2048 rows, 256 per core.
Host packs inputs per core: qT2 = [q^T; q^T] fp16 (q = feat@W1^T), kpair
fp32 columns [k[2p]; k[2p+1]] (k = feat@W2^T), feat in 128-row blocks with
a ones column (rowsum trick), transposed adj tiles, the a-weight slot
matrix A32s, and an identity for PE transposes.

Per-core algorithm (i = destination row, j = source node, d = head dim 64):
  scores[i, j] = sum_d a[d] * relu(q[j, d] + k[i, d])
E2 tile per row-pair: relu(qT2 + kpair[:, p]) on DVE (396ns, 4x mode) or
ACT (1061ns) -- split ~47/17 per 64-pair i-tile to balance both engines.
Scores via PE matmuls with lhsT = A32s slot q, 16 pairs accumulating into
a 32-row psum band; the two 512-col halves of a pair are emitted skewed
(s0 of pair idx, then s1 of pair idx-1) so consecutive matmuls hit
different PE column bands -- the PE streams up to 4 cols/cycle across 4
concurrent bands instead of serializing at 1 col/cycle.
Softmax: exp without row-max stabilizer (scores bounded); adj mask folded
into the post-transpose PSUM eviction (attT = pst * adjT).  The whole
i-tile tail (exp/transpose/mask/att-matmul/norm) is software-pipelined
into the NEXT i-tile's E2 stream so no engine drains at the boundary.
Final: out[i, :] = (att_unnorm @ [feat|1]) / rowsum.
"""
import sys

sys.path.insert(0, "/opt/trn_rl_repo")

from contextlib import ExitStack

import numpy as np

import concourse.bass as bass  # noqa: F401
import concourse.tile as tile
from concourse import bacc, mybir
from concourse.bass_utils import run_bass_kernel_spmd

B, T, C_IN, D = 2, 1024, 128, 64
N_CORES = 8
ROWS = (B * T) // N_CORES  # 256 destination rows per core
CPB = N_CORES // B  # cores per batch
NT = T // 128  # token tiles
NIT = ROWS // 128  # i-tiles per core
NPAIR = 64  # row pairs per i-tile
NSLOT = 16  # pair slots per 32-row psum band

FP32 = mybir.dt.float32
FP16 = mybir.dt.float16
OP = mybir.AluOpType
AF = mybir.ActivationFunctionType


def _on_scalar(idx):
    # 17 of 64 E2 tiles per i-tile on the ACT engine
    return idx % 4 == 2 or idx == 33


def _emit(ctx, tc, nc, hot_in, cold_in, adjT, out):
    singles = ctx.enter_context(tc.tile_pool(name="singles", bufs=1))
    hot = singles.tile([128, 2048], FP16)  # power-of-two row pitch
    cold = singles.tile([128, 128 + NT * (C_IN + 1)], FP16)
    dum = singles.tile([128, 2], FP16)
    qT2 = hot[:, 0:T]
    kpair16 = hot[:, T : T + ROWS // 2]
    A32s = hot[:, T + ROWS // 2 :]
    kpair = singles.tile([128, ROWS // 2], FP32)
    ident16 = cold[:, 0:128]
    feat16 = cold[:, 128:]

    # DMA triggers cost ~650ns serial per issuing sequencer: one hot DMA
    # first on sync, cold/adjT on the otherwise idle gpsimd sequencer.
    nc.sync.dma_start(hot[:], hot_in[:, :])
    adj_sb = []
    adjpool = ctx.enter_context(tc.tile_pool(name="adjp", bufs=NIT))
    for it in range(NIT):
        at = adjpool.tile([128, T], FP16, tag="adjT", name=f"adjT_{it}")
        adj_sb.append(at)

    # trigger the ACT table load (Exp) immediately, no data deps
    nc.scalar.memzero(dum[:])
    nc.scalar.activation(dum[:], dum[:], AF.Exp)
    # tensor_scalar needs an fp32 scalar: upconvert kpair once on ACT
    nc.scalar.copy(kpair[:], kpair16)

    e2pool = ctx.enter_context(tc.tile_pool(name="e2", bufs=8))
    softpool = ctx.enter_context(tc.tile_pool(name="soft", bufs=2))
    smallpool = ctx.enter_context(tc.tile_pool(name="small", bufs=2))
    attTpool = ctx.enter_context(tc.tile_pool(name="attT", bufs=2))
    outpool = ctx.enter_context(tc.tile_pool(name="outp", bufs=2))
    ps_scores = ctx.enter_context(tc.tile_pool(name="ps_s", bufs=2, space="PSUM"))
    ps_tr = ctx.enter_context(tc.tile_pool(name="ps_tr", bufs=2, space="PSUM"))
    ps_out = ctx.enter_context(tc.tile_pool(name="ps_o", bufs=2, space="PSUM"))

    W = C_IN + 1
    state = {}  # tail state of the previous i-tile

    def tail_exp(st, lo, step):
        nc.scalar.activation(
            st["pexp"][:, lo : lo + step], st["s"][:, lo : lo + step], AF.Exp
        )

    def tail_transpose(st, t):
        nc.tensor.transpose(
            st["pst"][:, t * 128 : (t + 1) * 128],
            st["pexp"][:, t * 128 : (t + 1) * 128],
            ident16,
        )

    def tail_mask(st, lo, step):
        nc.vector.tensor_tensor(
            st["attT"][:, lo : lo + step],
            st["pst"][:, lo : lo + step],
            adj_sb[st["it"]][:, lo : lo + step],
            OP.mult,
        )

    def tail_attmm(st, t):
        nc.tensor.matmul(
            st["po"][:],
            st["attT"][:, t * 128 : (t + 1) * 128],
            feat16[:, t * W : (t + 1) * W],
            start=(t == 0),
            stop=(t == NT - 1),
            skip_group_check=True,
        )

    def tail_norm_and_out(st):
        inv = smallpool.tile([128, 1], FP32, tag="inv")
        nc.vector.reciprocal(inv[:], st["po"][:, C_IN : C_IN + 1])
        out_sb = outpool.tile([128, C_IN], FP32, tag="out")
        nc.scalar.mul(out_sb[:], st["po"][:, 0:C_IN], inv[:])
        it = st["it"]
        nc.gpsimd.dma_start(out[it * 128 : (it + 1) * 128, :], out_sb[:])

    for it in range(NIT):
        s = ps_scores.tile([128, T], FP32, tag="s")  # 2 banks; halves by matmul
        prev = state if it > 0 else None
        pend = None  # (e2, lhsT, g, first, last) awaiting its 512:T half
        for idx in range(NPAIR):
            q, g = divmod(idx, 4)
            p = NSLOT * g + q
            P = it * NPAIR + p
            e2 = e2pool.tile([128, T], FP16, tag="e2")
            kcol = kpair[:, P : P + 1]
            if _on_scalar(idx):
                nc.scalar.activation(e2[:], qT2, AF.Relu, bias=kcol)
            else:
                nc.vector.tensor_scalar(e2[:], qT2, kcol, 0.0, OP.add, OP.max)
            lhsT = A32s[:, 32 * q : 32 * q + 32]
            first, last = q == 0, q == NSLOT - 1
            nc.tensor.matmul(
                s[32 * g : 32 * g + 32, 0:512],
                lhsT,
                e2[:, 0:512],
                start=first,
                stop=last,
                tile_position=(0, 32 * g),
                skip_group_check=True,
            )
            if pend is not None:
                pe2, plhsT, pg, pfirst, plast = pend
                nc.tensor.matmul(
                    s[32 * pg : 32 * pg + 32, 512:T],
                    plhsT,
                    pe2[:, 512:T],
                    start=pfirst,
                    stop=plast,
                    tile_position=(0, 32 * pg),
                    skip_group_check=True,
                )
            pend = (e2, lhsT, g, first, last)

            # cold inputs (identity/feat blocks/adj): trigger their DMAs
            # after the E2 stream is underway -- data needed from i-tile 1 on
            if it == 0:
                if idx == 8:
                    nc.gpsimd.dma_start(cold[:], cold_in[:, :])
                elif idx == 12:
                    nc.gpsimd.dma_start(adj_sb[0][:], adjT[:, 0:T])
                elif idx == 16:
                    nc.gpsimd.dma_start(adj_sb[1][:], adjT[:, T : 2 * T])

            # software-pipelined tail of the previous i-tile
            if prev is not None:
                if idx == 4:
                    tail_exp(prev, 0, T)
                elif 6 <= idx <= 13:
                    tail_transpose(prev, idx - 6)
                elif idx == 15:
                    tail_mask(prev, 0, T)
                elif 17 <= idx <= 24:
                    tail_attmm(prev, idx - 17)
                elif idx == 26:
                    tail_norm_and_out(prev)

        pe2, plhsT, pg, pfirst, plast = pend
        nc.tensor.matmul(
            s[32 * pg : 32 * pg + 32, 512:T],
            plhsT,
            pe2[:, 512:T],
            start=pfirst,
            stop=plast,
            tile_position=(0, 32 * pg),
            skip_group_check=True,
        )

        state = {
            "it": it,
            "s": s,
            "pexp": softpool.tile([128, T], FP16, tag="pexp", name=f"pexp_{it}"),
            "pst": ps_tr.tile([128, T], FP16, tag="tr", name=f"pst_{it}"),
            "attT": attTpool.tile([128, T], FP16, tag="attT", name=f"attT_{it}"),
            "po": ps_out.tile([128, W], FP32, tag="o", name=f"po_{it}"),
        }

    # final i-tile's tail, pipelined in j-quarters to shorten the exposed chain
    st = state
    for hh in range(4):
        lo = hh * 256
        tail_exp(st, lo, 256)
        for t in range(lo // 128, (lo + 256) // 128):
            tail_transpose(st, t)
        tail_mask(st, lo, 256)
        for t in range(lo // 128, (lo + 256) // 128):
            tail_attmm(st, t)
    tail_norm_and_out(st)


_PROGRAM = None


def build_program():
    global _PROGRAM
    if _PROGRAM is not None:
        return _PROGRAM
    nc = bacc.Bacc("TRN2", target_bir_lowering=False, debug=False, num_devices=N_CORES)
    hot_in = nc.dram_tensor("hot", [128, 2048], FP16, kind="ExternalInput")
    cold_in = nc.dram_tensor("cold", [128, 128 + NT * (C_IN + 1)], FP16, kind="ExternalInput")
    adjT = nc.dram_tensor("adjT", [128, NIT * T], FP16, kind="ExternalInput")
    out = nc.dram_tensor("out", [ROWS, C_IN], FP32, kind="ExternalOutput")
    with tile.TileContext(nc) as tc:
        with ExitStack() as ctx:
            _emit(ctx, tc, nc, hot_in, cold_in, adjT, out)
    nc.compile()
    _PROGRAM = nc
    return nc


def make_a32(a):
    a32 = np.zeros((128, NSLOT * 32), dtype=np.float16)
    for q in range(NSLOT):
        a32[0:64, 32 * q + 2 * q] = a
        a32[64:128, 32 * q + 2 * q + 1] = a
    return a32


def make_in_maps(feat, adj, W1, W2, a):
    feat = np.ascontiguousarray(feat, dtype=np.float32)
    adj = np.ascontiguousarray(adj, dtype=np.float32)
    W1 = np.asarray(W1, dtype=np.float32)
    W2 = np.asarray(W2, dtype=np.float32)
    a32 = make_a32(np.asarray(a, dtype=np.float32))
    ident = np.eye(128, dtype=np.float16)
    in_maps = []
    for b in range(B):
        feat16 = feat[b].astype(np.float16)  # [T, C_IN]
        q = (feat16.astype(np.float32) @ W1.T).astype(np.float16)  # [T, D]
        k = feat16.astype(np.float32) @ W2.T  # [T, D] fp32
        qT = np.ascontiguousarray(q.T)  # [D, T]
        qT2 = np.ascontiguousarray(np.concatenate([qT, qT], axis=0))  # [128, T]
        fb = feat16.reshape(NT, 128, C_IN).transpose(1, 0, 2)  # [128, NT, C_IN]
        fblk = np.concatenate(
            [fb, np.ones((128, NT, 1), dtype=np.float16)], axis=2
        ).reshape(128, NT * (C_IN + 1))
        fblk = np.ascontiguousarray(fblk)
        for cc in range(CPB):
            r0 = cc * ROWS
            krows = k[r0 : r0 + ROWS].astype(np.float32)  # [256, 64]
            kp = krows.reshape(ROWS // 2, 2, D)  # [p, two, d]
            kpair = np.ascontiguousarray(
                kp.transpose(1, 2, 0).reshape(128, ROWS // 2).astype(np.float32)
            )  # [[k2p d]; [k2p+1 d]] stacked -> [128, 128]
            arows = adj[b, r0 : r0 + ROWS].astype(np.float16)  # [256, 1024]
            ats = []
            for it in range(NIT):
                chunk = arows[it * 128 : (it + 1) * 128]  # [128 i, 1024 j]
                x = chunk.reshape(128, NT, 128)  # [ii, t, p]
                ats.append(x.transpose(2, 1, 0).reshape(128, T))  # [p, (t, ii)]
            adjT = np.ascontiguousarray(np.concatenate(ats, axis=1))
            hot = np.zeros((128, 2048), dtype=np.float16)
            hot[:, 0:T] = qT2
            hot[:, T : T + ROWS // 2] = kpair.astype(np.float16)
            hot[:, T + ROWS // 2 : T + ROWS // 2 + NSLOT * 32] = a32
            cold = np.ascontiguousarray(np.concatenate([ident, fblk], axis=1))
            in_maps.append({"hot": hot, "cold": cold, "adjT": adjT})
    return in_maps


def run(feat, adj, W1, W2, a, trace=False):
    nc = build_program()
    in_maps = make_in_maps(feat, adj, W1, W2, a)
    last_err = None
    for attempt in range(3):
        try:
            res = run_bass_kernel_spmd(
                nc, in_maps, core_ids=list(range(N_CORES)), trace=trace
            )
            outs = [np.asarray(res.results[c]["out"]) for c in range(N_CORES)]
            break
        except Exception as e:  # transient NRT device errors recover on retry
            last_err = e
            import time

            time.sleep(5)
    else:
        raise last_err
    full = np.concatenate(outs, axis=0).reshape(B, T, C_IN).astype(np.float32)
    return full, res


def kernel(feat, adj, W1, W2, a):
    full, _ = run(feat, adj, W1, W2, a)
    return full


# revision 10
# speedup vs baseline: 1.1584x; 1.0086x over previous
"""GATv2 attention layer (B=2, T=1024, C_IN=128, D=64) on 8 trn2 NeuronCores.

Sharding: flatten (B, T) destination rows -> 2048 rows, 256 per core.
Host packs inputs per core: qT2 = [q^T; q^T] fp16 (q = feat@W1^T), kpair
fp32 columns [k[2p]; k[2p+1]] (k = feat@W2^T), feat in 128-row blocks with
a ones column (rowsum trick), transposed adj tiles, the a-weight slot
matrix A32s, and an identity for PE transposes.

Per-core algorithm (i = destination row, j = source node, d = head dim 64):
  scores[i, j] = sum_d a[d] * relu(q[j, d] + k[i, d])
E2 tile per row-pair: relu(qT2 + kpair[:, p]) on DVE (396ns, 4x mode) or
ACT (1061ns) -- split ~47/17 per 64-pair i-tile to balance both engines.
Scores via PE matmuls with lhsT = A32s slot q, 16 pairs accumulating into
a 32-row psum band; the two 512-col halves of a pair are emitted skewed
(s0 of pair idx, then s1 of pair idx-1) so consecutive matmuls hit
different PE column bands -- the PE streams up to 4 cols/cycle across 4
concurrent bands instead of serializing at 1 col/cycle.
Softmax: exp without row-max stabilizer (scores bounded); adj mask folded
into the post-transpose PSUM eviction (attT = pst * adjT).  The whole
i-tile tail (exp/transpose/mask/att-matmul/norm) is software-pipelined
into the NEXT i-tile's E2 stream so no engine drains at the boundary.
Final: out[i, :] = (att_unnorm @ [feat|1]) / rowsum.
"""
import sys

sys.path.insert(0, "/opt/trn_rl_repo")

from contextlib import ExitStack

import numpy as np

import concourse.bass as bass  # noqa: F401
import concourse.tile as tile
from concourse import bacc, mybir
from concourse.bass_utils import run_bass_kernel_spmd

B, T, C_IN, D = 2, 1024, 128, 64
N_CORES = 8
ROWS = (B * T) // N_CORES  # 256 destination rows per core
CPB = N_CORES // B  # cores per batch
NT = T // 128  # token tiles
NIT = ROWS // 128  # i-tiles per core
NPAIR = 64  # row pairs per i-tile
NSLOT = 16  # pair slots per 32-row psum band

FP32 = mybir.dt.float32
FP16 = mybir.dt.float16
OP = mybir.AluOpType
AF = mybir.ActivationFunctionType


def _on_scalar(idx):
    # 17 of 64 E2 tiles per i-tile on the ACT engine
    return idx % 4 == 2 or idx == 33


def _emit(ctx, tc, nc, hot_in, cold_in, adjT, out):
    singles = ctx.enter_context(tc.tile_pool(name="singles", bufs=1))
    hot = singles.tile([128, 2048], FP16)  # power-of-two row pitch
    cold = singles.tile([128, 128 + NT * (C_IN + 1)], FP16)
    dum = singles.tile([128, 2], FP16)
    qT2 = hot[:, 0:T]
    kpair16 = hot[:, T : T + ROWS // 2]
    A32s = hot[:, T + ROWS // 2 :]
    kpair = singles.tile([128, ROWS // 2], FP32)
    ident16 = cold[:, 0:128]
    feat16 = cold[:, 128:]

    # DMA triggers cost ~650ns serial per issuing sequencer: one hot DMA
    # first on sync, cold/adjT on the otherwise idle gpsimd sequencer.
    # one dma_start = one DMA queue ring: split the hot tile across two
    # triggers on two idle sequencers for queue parallelism
    nc.sync.dma_start(hot[:, 0:T], hot_in[:, 0:T])
    nc.gpsimd.dma_start(hot[:, T:2048], hot_in[:, T:2048])
    adj_sb = []
    adjpool = ctx.enter_context(tc.tile_pool(name="adjp", bufs=NIT))
    for it in range(NIT):
        at = adjpool.tile([128, T], FP16, tag="adjT", name=f"adjT_{it}")
        adj_sb.append(at)

    # trigger the ACT table load (Exp) immediately, no data deps
    nc.scalar.memzero(dum[:])
    nc.scalar.activation(dum[:], dum[:], AF.Exp)
    # tensor_scalar needs an fp32 scalar: upconvert kpair once on ACT
    nc.scalar.copy(kpair[:], kpair16)

    e2pool = ctx.enter_context(tc.tile_pool(name="e2", bufs=8))
    softpool = ctx.enter_context(tc.tile_pool(name="soft", bufs=2))
    smallpool = ctx.enter_context(tc.tile_pool(name="small", bufs=2))
    attTpool = ctx.enter_context(tc.tile_pool(name="attT", bufs=2))
    outpool = ctx.enter_context(tc.tile_pool(name="outp", bufs=2))
    ps_scores = ctx.enter_context(tc.tile_pool(name="ps_s", bufs=2, space="PSUM"))
    ps_tr = ctx.enter_context(tc.tile_pool(name="ps_tr", bufs=2, space="PSUM"))
    ps_out = ctx.enter_context(tc.tile_pool(name="ps_o", bufs=2, space="PSUM"))

    W = C_IN + 1
    state = {}  # tail state of the previous i-tile

    def tail_exp(st, lo, step):
        nc.scalar.activation(
            st["pexp"][:, lo : lo + step], st["s"][:, lo : lo + step], AF.Exp
        )

    def tail_transpose(st, t):
        nc.tensor.transpose(
            st["pst"][:, t * 128 : (t + 1) * 128],
            st["pexp"][:, t * 128 : (t + 1) * 128],
            ident16,
        )

    def tail_mask(st, lo, step):
        nc.vector.tensor_tensor(
            st["attT"][:, lo : lo + step],
            st["pst"][:, lo : lo + step],
            adj_sb[st["it"]][:, lo : lo + step],
            OP.mult,
        )

    def tail_attmm(st, t):
        nc.tensor.matmul(
            st["po"][:],
            st["attT"][:, t * 128 : (t + 1) * 128],
            feat16[:, t * W : (t + 1) * W],
            start=(t == 0),
            stop=(t == NT - 1),
            skip_group_check=True,
        )

    def tail_norm_and_out(st):
        inv = smallpool.tile([128, 1], FP32, tag="inv")
        nc.vector.reciprocal(inv[:], st["po"][:, C_IN : C_IN + 1])
        out_sb = outpool.tile([128, C_IN], FP32, tag="out")
        nc.scalar.mul(out_sb[:], st["po"][:, 0:C_IN], inv[:])
        it = st["it"]
        nc.gpsimd.dma_start(out[it * 128 : (it + 1) * 128, :], out_sb[:])

    for it in range(NIT):
        s = ps_scores.tile([128, T], FP32, tag="s")  # 2 banks; halves by matmul
        prev = state if it > 0 else None
        pend = None  # (e2, lhsT, g, first, last) awaiting its 512:T half
        for idx in range(NPAIR):
            q, g = divmod(idx, 4)
            p = NSLOT * g + q
            P = it * NPAIR + p
            e2 = e2pool.tile([128, T], FP16, tag="e2")
            kcol = kpair[:, P : P + 1]
            if _on_scalar(idx):
                nc.scalar.activation(e2[:], qT2, AF.Relu, bias=kcol)
            else:
                nc.vector.tensor_scalar(e2[:], qT2, kcol, 0.0, OP.add, OP.max)
            lhsT = A32s[:, 32 * q : 32 * q + 32]
            first, last = q == 0, q == NSLOT - 1
            nc.tensor.matmul(
                s[32 * g : 32 * g + 32, 0:512],
                lhsT,
                e2[:, 0:512],
                start=first,
                stop=last,
                tile_position=(0, 32 * g),
                skip_group_check=True,
            )
            if pend is not None:
                pe2, plhsT, pg, pfirst, plast = pend
                nc.tensor.matmul(
                    s[32 * pg : 32 * pg + 32, 512:T],
                    plhsT,
                    pe2[:, 512:T],
                    start=pfirst,
                    stop=plast,
                    tile_position=(0, 32 * pg),
                    skip_group_check=True,
                )
            pend = (e2, lhsT, g, first, last)

            # cold inputs (identity/feat blocks/adj): trigger their DMAs
            # after the E2 stream is underway -- data needed from i-tile 1 on
            if it == 0:
                if idx == 8:
                    nc.gpsimd.dma_start(cold[:], cold_in[:, :])
                elif idx == 12:
                    nc.gpsimd.dma_start(adj_sb[0][:], adjT[:, 0:T])
                elif idx == 16:
                    nc.gpsimd.dma_start(adj_sb[1][:], adjT[:, T : 2 * T])

            # software-pipelined tail of the previous i-tile
            if prev is not None:
                if idx == 4:
                    tail_exp(prev, 0, T)
                elif 6 <= idx <= 13:
                    tail_transpose(prev, idx - 6)
                elif idx == 15:
                    tail_mask(prev, 0, T)
                elif 17 <= idx <= 24:
                    tail_attmm(prev, idx - 17)
                elif idx == 26:
                    tail_norm_and_out(prev)

        pe2, plhsT, pg, pfirst, plast = pend
        nc.tensor.matmul(
            s[32 * pg : 32 * pg + 32, 512:T],
            plhsT,
            pe2[:, 512:T],
            start=pfirst,
            stop=plast,
            tile_position=(0, 32 * pg),
            skip_group_check=True,
        )

        state = {
            "it": it,
            "s": s,
            "pexp": softpool.tile([128, T], FP16, tag="pexp", name=f"pexp_{it}"),
            "pst": ps_tr.tile([128, T], FP16, tag="tr", name=f"pst_{it}"),
            "attT": attTpool.tile([128, T], FP16, tag="attT", name=f"attT_{it}"),
            "po": ps_out.tile([128, W], FP32, tag="o", name=f"po_{it}"),
        }

    # final i-tile's tail, pipelined in j-quarters to shorten the exposed chain
    st = state
    for hh in range(4):
        lo = hh * 256
        tail_exp(st, lo, 256)
        for t in range(lo // 128, (lo + 256) // 128):
            tail_transpose(st, t)
        tail_mask(st, lo, 256)
        for t in range(lo // 128, (lo + 256) // 128):
            tail_attmm(st, t)
    tail_norm_and_out(st)


_PROGRAM = None


def build_program():
    global _PROGRAM
    if _PROGRAM is not None:
        return _PROGRAM
    nc = bacc.Bacc("TRN2", target_bir_lowering=False, debug=False, num_devices=N_CORES)
    hot_in = nc.dram_tensor("hot", [128, 2048], FP16, kind="ExternalInput")
    cold_in = nc.dram_tensor("cold", [128, 128 + NT * (C_IN + 1)], FP16, kind="ExternalInput")
    adjT = nc.dram_tensor("adjT", [128, NIT * T], FP16, kind="ExternalInput")
    out = nc.dram_tensor("out", [ROWS, C_IN], FP32, kind="ExternalOutput")
    with tile.TileContext(nc) as tc:
        with ExitStack() as ctx:
            _emit(ctx, tc, nc, hot_in, cold_in, adjT, out)
    nc.compile()
    _PROGRAM = nc
    return nc


def make_a32(a):
    a32 = np.zeros((128, NSLOT * 32), dtype=np.float16)
    for q in range(NSLOT):
        a32[0:64, 32 * q + 2 * q] = a
        a32[64:128, 32 * q + 2 * q + 1] = a
    return a32


def make_in_maps(feat, adj, W1, W2, a):
    feat = np.ascontiguousarray(feat, dtype=np.float32)
    adj = np.ascontiguousarray(adj, dtype=np.float32)
    W1 = np.asarray(W1, dtype=np.float32)
    W2 = np.asarray(W2, dtype=np.float32)
    a32 = make_a32(np.asarray(a, dtype=np.float32))
    ident = np.eye(128, dtype=np.float16)
    in_maps = []
    for b in range(B):
        feat16 = feat[b].astype(np.float16)  # [T, C_IN]
        q = (feat16.astype(np.float32) @ W1.T).astype(np.float16)  # [T, D]
        k = feat16.astype(np.float32) @ W2.T  # [T, D] fp32
        qT = np.ascontiguousarray(q.T)  # [D, T]
        qT2 = np.ascontiguousarray(np.concatenate([qT, qT], axis=0))  # [128, T]
        fb = feat16.reshape(NT, 128, C_IN).transpose(1, 0, 2)  # [128, NT, C_IN]
        fblk = np.concatenate(
            [fb, np.ones((128, NT, 1), dtype=np.float16)], axis=2
        ).reshape(128, NT * (C_IN + 1))
        fblk = np.ascontiguousarray(fblk)
        for cc in range(CPB):
            r0 = cc * ROWS
            krows = k[r0 : r0 + ROWS].astype(np.float32)  # [256, 64]
            kp = krows.reshape(ROWS // 2, 2, D)  # [p, two, d]
            kpair = np.ascontiguousarray(
                kp.transpose(1, 2, 0).reshape(128, ROWS // 2).astype(np.float32)
            )  # [[k2p d]; [k2p+1 d]] stacked -> [128, 128]
            arows = adj[b, r0 : r0 + ROWS].astype(np.float16)  # [256, 1024]
            ats = []
            for it in range(NIT):
                chunk = arows[it * 128 : (it + 1) * 128]  # [128 i, 1024 j]
                x = chunk.reshape(128, NT, 128)  # [ii, t, p]
                ats.append(x.transpose(2, 1, 0).reshape(128, T))  # [p, (t, ii)]
            adjT = np.ascontiguousarray(np.concatenate(ats, axis=1))
            hot = np.zeros((128, 2048), dtype=np.float16)
            hot[:, 0:T] = qT2
            hot[:, T : T + ROWS // 2] = kpair.astype(np.float16)
            hot[:, T + ROWS // 2 : T + ROWS // 2 + NSLOT * 32] = a32
            cold = np.ascontiguousarray(np.concatenate([ident, fblk], axis=1))
            in_maps.append({"hot": hot, "cold": cold, "adjT": adjT})
    return in_maps


def run(feat, adj, W1, W2, a, trace=False):
    nc = build_program()
    in_maps = make_in_maps(feat, adj, W1, W2, a)
    last_err = None
    for attempt in range(3):
        try:
            res = run_bass_kernel_spmd(
                nc, in_maps, core_ids=list(range(N_CORES)), trace=trace
            )
            outs = [np.asarray(res.results[c]["out"]) for c in range(N_CORES)]
            break
        except Exception as e:  # transient NRT device errors recover on retry
            last_err = e
            import time

            time.sleep(5)
    else:
        raise last_err
    full = np.concatenate(outs, axis=0).reshape(B, T, C_IN).astype(np.float32)
    return full, res


def kernel(feat, adj, W1, W2, a):
    full, _ = run(feat, adj, W1, W2, a)
    return full


# revision 11
# speedup vs baseline: 1.1587x; 1.0002x over previous
"""GATv2 attention layer (B=2, T=1024, C_IN=128, D=64) on 8 trn2 NeuronCores.

Sharding: flatten (B, T) destination rows -> 2048 rows, 256 per core.
Host packs inputs per core: qT2 = [q^T; q^T] fp16 (q = feat@W1^T), kpair
fp32 columns [k[2p]; k[2p+1]] (k = feat@W2^T), feat in 128-row blocks with
a ones column (rowsum trick), transposed adj tiles, the a-weight slot
matrix A32s, and an identity for PE transposes.

Per-core algorithm (i = destination row, j = source node, d = head dim 64):
  scores[i, j] = sum_d a[d] * relu(q[j, d] + k[i, d])
E2 tile per row-pair: relu(qT2 + kpair[:, p]) on DVE (396ns, 4x mode) or
ACT (1061ns) -- split ~47/17 per 64-pair i-tile to balance both engines.
Scores via PE matmuls with lhsT = A32s slot q, 16 pairs accumulating into
a 32-row psum band; the two 512-col halves of a pair are emitted skewed
(s0 of pair idx, then s1 of pair idx-1) so consecutive matmuls hit
different PE column bands -- the PE streams up to 4 cols/cycle across 4
concurrent bands instead of serializing at 1 col/cycle.
Softmax: exp without row-max stabilizer (scores bounded); adj mask folded
into the post-transpose PSUM eviction (attT = pst * adjT).  The whole
i-tile tail (exp/transpose/mask/att-matmul/norm) is software-pipelined
into the NEXT i-tile's E2 stream so no engine drains at the boundary.
Final: out[i, :] = (att_unnorm @ [feat|1]) / rowsum.
"""
import sys

sys.path.insert(0, "/opt/trn_rl_repo")

from contextlib import ExitStack

import numpy as np

import concourse.bass as bass  # noqa: F401
import concourse.tile as tile
from concourse import bacc, mybir
from concourse.bass_utils import run_bass_kernel_spmd

B, T, C_IN, D = 2, 1024, 128, 64
N_CORES = 8
ROWS = (B * T) // N_CORES  # 256 destination rows per core
CPB = N_CORES // B  # cores per batch
NT = T // 128  # token tiles
NIT = ROWS // 128  # i-tiles per core
NPAIR = 64  # row pairs per i-tile
NSLOT = 16  # pair slots per 32-row psum band

FP32 = mybir.dt.float32
FP16 = mybir.dt.float16
OP = mybir.AluOpType
AF = mybir.ActivationFunctionType


def _on_scalar(idx):
    # 17 of 64 E2 tiles per i-tile on the ACT engine
    return idx % 4 == 2 or idx == 33


def _emit(ctx, tc, nc, hot_in, cold_in, adjT, out):
    singles = ctx.enter_context(tc.tile_pool(name="singles", bufs=1))
    hot = singles.tile([128, 2048], FP16)  # power-of-two row pitch
    cold = singles.tile([128, 128 + NT * (C_IN + 1)], FP16)
    dum = singles.tile([128, 2], FP16)
    qT2 = hot[:, 0:T]
    kpair16 = hot[:, T : T + ROWS // 2]
    A32s = hot[:, T + ROWS // 2 :]
    kpair = singles.tile([128, ROWS // 2], FP32)
    ident16 = cold[:, 0:128]
    feat16 = cold[:, 128:]

    # DMA triggers cost ~650ns serial per issuing sequencer: one hot DMA
    # first on sync, cold/adjT on the otherwise idle gpsimd sequencer.
    # one dma_start = one DMA queue ring: split the hot tile across three
    # triggers on three sequencers (parallel trigger + ring parallelism)
    nc.sync.dma_start(hot[:, 0:512], hot_in[:, 0:512])
    nc.scalar.dma_start(hot[:, 512:T], hot_in[:, 512:T])
    nc.gpsimd.dma_start(hot[:, T:2048], hot_in[:, T:2048])
    adj_sb = []
    adjpool = ctx.enter_context(tc.tile_pool(name="adjp", bufs=NIT))
    for it in range(NIT):
        at = adjpool.tile([128, T], FP16, tag="adjT", name=f"adjT_{it}")
        adj_sb.append(at)

    # trigger the ACT table load (Exp) immediately, no data deps
    nc.scalar.memzero(dum[:])
    nc.scalar.activation(dum[:], dum[:], AF.Exp)
    # tensor_scalar needs an fp32 scalar: upconvert kpair once on ACT
    nc.scalar.copy(kpair[:], kpair16)

    e2pool = ctx.enter_context(tc.tile_pool(name="e2", bufs=8))
    softpool = ctx.enter_context(tc.tile_pool(name="soft", bufs=2))
    smallpool = ctx.enter_context(tc.tile_pool(name="small", bufs=2))
    attTpool = ctx.enter_context(tc.tile_pool(name="attT", bufs=2))
    outpool = ctx.enter_context(tc.tile_pool(name="outp", bufs=2))
    ps_scores = ctx.enter_context(tc.tile_pool(name="ps_s", bufs=2, space="PSUM"))
    ps_tr = ctx.enter_context(tc.tile_pool(name="ps_tr", bufs=2, space="PSUM"))
    ps_out = ctx.enter_context(tc.tile_pool(name="ps_o", bufs=2, space="PSUM"))

    W = C_IN + 1
    state = {}  # tail state of the previous i-tile

    def tail_exp(st, lo, step):
        nc.scalar.activation(
            st["pexp"][:, lo : lo + step], st["s"][:, lo : lo + step], AF.Exp
        )

    def tail_transpose(st, t):
        nc.tensor.transpose(
            st["pst"][:, t * 128 : (t + 1) * 128],
            st["pexp"][:, t * 128 : (t + 1) * 128],
            ident16,
        )

    def tail_mask(st, lo, step):
        nc.vector.tensor_tensor(
            st["attT"][:, lo : lo + step],
            st["pst"][:, lo : lo + step],
            adj_sb[st["it"]][:, lo : lo + step],
            OP.mult,
        )

    def tail_attmm(st, t):
        nc.tensor.matmul(
            st["po"][:],
            st["attT"][:, t * 128 : (t + 1) * 128],
            feat16[:, t * W : (t + 1) * W],
            start=(t == 0),
            stop=(t == NT - 1),
            skip_group_check=True,
        )

    def tail_norm_and_out(st):
        inv = smallpool.tile([128, 1], FP32, tag="inv")
        nc.vector.reciprocal(inv[:], st["po"][:, C_IN : C_IN + 1])
        out_sb = outpool.tile([128, C_IN], FP32, tag="out")
        nc.scalar.mul(out_sb[:], st["po"][:, 0:C_IN], inv[:])
        it = st["it"]
        nc.gpsimd.dma_start(out[it * 128 : (it + 1) * 128, :], out_sb[:])

    for it in range(NIT):
        s = ps_scores.tile([128, T], FP32, tag="s")  # 2 banks; halves by matmul
        prev = state if it > 0 else None
        pend = None  # (e2, lhsT, g, first, last) awaiting its 512:T half
        for idx in range(NPAIR):
            q, g = divmod(idx, 4)
            p = NSLOT * g + q
            P = it * NPAIR + p
            e2 = e2pool.tile([128, T], FP16, tag="e2")
            kcol = kpair[:, P : P + 1]
            if _on_scalar(idx):
                nc.scalar.activation(e2[:], qT2, AF.Relu, bias=kcol)
            else:
                nc.vector.tensor_scalar(e2[:], qT2, kcol, 0.0, OP.add, OP.max)
            lhsT = A32s[:, 32 * q : 32 * q + 32]
            first, last = q == 0, q == NSLOT - 1
            nc.tensor.matmul(
                s[32 * g : 32 * g + 32, 0:512],
                lhsT,
                e2[:, 0:512],
                start=first,
                stop=last,
                tile_position=(0, 32 * g),
                skip_group_check=True,
            )
            if pend is not None:
                pe2, plhsT, pg, pfirst, plast = pend
                nc.tensor.matmul(
                    s[32 * pg : 32 * pg + 32, 512:T],
                    plhsT,
                    pe2[:, 512:T],
                    start=pfirst,
                    stop=plast,
                    tile_position=(0, 32 * pg),
                    skip_group_check=True,
                )
            pend = (e2, lhsT, g, first, last)

            # cold inputs (identity/feat blocks/adj): trigger their DMAs
            # after the E2 stream is underway -- data needed from i-tile 1 on
            if it == 0:
                if idx == 8:
                    nc.gpsimd.dma_start(cold[:], cold_in[:, :])
                elif idx == 12:
                    nc.gpsimd.dma_start(adj_sb[0][:], adjT[:, 0:T])
                elif idx == 16:
                    nc.gpsimd.dma_start(adj_sb[1][:], adjT[:, T : 2 * T])

            # software-pipelined tail of the previous i-tile
            if prev is not None:
                if idx == 4:
                    tail_exp(prev, 0, T)
                elif 6 <= idx <= 13:
                    tail_transpose(prev, idx - 6)
                elif idx == 15:
                    tail_mask(prev, 0, T)
                elif 17 <= idx <= 24:
                    tail_attmm(prev, idx - 17)
                elif idx == 26:
                    tail_norm_and_out(prev)

        pe2, plhsT, pg, pfirst, plast = pend
        nc.tensor.matmul(
            s[32 * pg : 32 * pg + 32, 512:T],
            plhsT,
            pe2[:, 512:T],
            start=pfirst,
            stop=plast,
            tile_position=(0, 32 * pg),
            skip_group_check=True,
        )

        state = {
            "it": it,
            "s": s,
            "pexp": softpool.tile([128, T], FP16, tag="pexp", name=f"pexp_{it}"),
            "pst": ps_tr.tile([128, T], FP16, tag="tr", name=f"pst_{it}"),
            "attT": attTpool.tile([128, T], FP16, tag="attT", name=f"attT_{it}"),
            "po": ps_out.tile([128, W], FP32, tag="o", name=f"po_{it}"),
        }

    # final i-tile's tail, pipelined in j-quarters to shorten the exposed chain
    st = state
    for hh in range(4):
        lo = hh * 256
        tail_exp(st, lo, 256)
        for t in range(lo // 128, (lo + 256) // 128):
            tail_transpose(st, t)
        tail_mask(st, lo, 256)
        for t in range(lo // 128, (lo + 256) // 128):
            tail_attmm(st, t)
    tail_norm_and_out(st)


_PROGRAM = None


def build_program():
    global _PROGRAM
    if _PROGRAM is not None:
        return _PROGRAM
    nc = bacc.Bacc("TRN2", target_bir_lowering=False, debug=False, num_devices=N_CORES)
    hot_in = nc.dram_tensor("hot", [128, 2048], FP16, kind="ExternalInput")
    cold_in = nc.dram_tensor("cold", [128, 128 + NT * (C_IN + 1)], FP16, kind="ExternalInput")
    adjT = nc.dram_tensor("adjT", [128, NIT * T], FP16, kind="ExternalInput")
    out = nc.dram_tensor("out", [ROWS, C_IN], FP32, kind="ExternalOutput")
    with tile.TileContext(nc) as tc:
        with ExitStack() as ctx:
            _emit(ctx, tc, nc, hot_in, cold_in, adjT, out)
    nc.compile()
    _PROGRAM = nc
    return nc


def make_a32(a):
    a32 = np.zeros((128, NSLOT * 32), dtype=np.float16)
    for q in range(NSLOT):
        a32[0:64, 32 * q + 2 * q] = a
        a32[64:128, 32 * q + 2 * q + 1] = a
    return a32


def make_in_maps(feat, adj, W1, W2, a):
    feat = np.ascontiguousarray(feat, dtype=np.float32)
    adj = np.ascontiguousarray(adj, dtype=np.float32)
    W1 = np.asarray(W1, dtype=np.float32)
    W2 = np.asarray(W2, dtype=np.float32)
    a32 = make_a32(np.asarray(a, dtype=np.float32))
    ident = np.eye(128, dtype=np.float16)
    in_maps = []
    for b in range(B):
        feat16 = feat[b].astype(np.float16)  # [T, C_IN]
        q = (feat16.astype(np.float32) @ W1.T).astype(np.float16)  # [T, D]
        k = feat16.astype(np.float32) @ W2.T  # [T, D] fp32
        qT = np.ascontiguousarray(q.T)  # [D, T]
        qT2 = np.ascontiguousarray(np.concatenate([qT, qT], axis=0))  # [128, T]
        fb = feat16.reshape(NT, 128, C_IN).transpose(1, 0, 2)  # [128, NT, C_IN]
        fblk = np.concatenate(
            [fb, np.ones((128, NT, 1), dtype=np.float16)], axis=2
        ).reshape(128, NT * (C_IN + 1))
        fblk = np.ascontiguousarray(fblk)
        for cc in range(CPB):
            r0 = cc * ROWS
            krows = k[r0 : r0 + ROWS].astype(np.float32)  # [256, 64]
            kp = krows.reshape(ROWS // 2, 2, D)  # [p, two, d]
            kpair = np.ascontiguousarray(
                kp.transpose(1, 2, 0).reshape(128, ROWS // 2).astype(np.float32)
            )  # [[k2p d]; [k2p+1 d]] stacked -> [128, 128]
            arows = adj[b, r0 : r0 + ROWS].astype(np.float16)  # [256, 1024]
            ats = []
            for it in range(NIT):
                chunk = arows[it * 128 : (it + 1) * 128]  # [128 i, 1024 j]
                x = chunk.reshape(128, NT, 128)  # [ii, t, p]
                ats.append(x.transpose(2, 1, 0).reshape(128, T))  # [p, (t, ii)]
            adjT = np.ascontiguousarray(np.concatenate(ats, axis=1))
            hot = np.zeros((128, 2048), dtype=np.float16)
            hot[:, 0:T] = qT2
            hot[:, T : T + ROWS // 2] = kpair.astype(np.float16)
            hot[:, T + ROWS // 2 : T + ROWS // 2 + NSLOT * 32] = a32
            cold = np.ascontiguousarray(np.concatenate([ident, fblk], axis=1))
            in_maps.append({"hot": hot, "cold": cold, "adjT": adjT})
    return in_maps


def run(feat, adj, W1, W2, a, trace=False):
    nc = build_program()
    in_maps = make_in_maps(feat, adj, W1, W2, a)
    last_err = None
    for attempt in range(3):
        try:
            res = run_bass_kernel_spmd(
                nc, in_maps, core_ids=list(range(N_CORES)), trace=trace
            )
            outs = [np.asarray(res.results[c]["out"]) for c in range(N_CORES)]
            break
        except Exception as e:  # transient NRT device errors recover on retry
            last_err = e
            import time

            time.sleep(5)
    else:
        raise last_err
    full = np.concatenate(outs, axis=0).reshape(B, T, C_IN).astype(np.float32)
    return full, res


def kernel(feat, adj, W1, W2, a):
    full, _ = run(feat, adj, W1, W2, a)
    return full


# revision 12
# speedup vs baseline: 1.1637x; 1.0043x over previous
"""GATv2 attention layer (B=2, T=1024, C_IN=128, D=64) on 8 trn2 NeuronCores.

Sharding: flatten (B, T) destination rows -> 2048 rows, 256 per core.
Host packs inputs per core: qT2 = [q^T; q^T] fp16 (q = feat@W1^T), kpair
fp32 columns [k[2p]; k[2p+1]] (k = feat@W2^T), feat in 128-row blocks with
a ones column (rowsum trick), transposed adj tiles, the a-weight slot
matrix A32s, and an identity for PE transposes.

Per-core algorithm (i = destination row, j = source node, d = head dim 64):
  scores[i, j] = sum_d a[d] * relu(q[j, d] + k[i, d])
E2 tile per row-pair: relu(qT2 + kpair[:, p]) on DVE (396ns, 4x mode) or
ACT (1061ns) -- split ~47/17 per 64-pair i-tile to balance both engines.
Scores via PE matmuls with lhsT = A32s slot q, 16 pairs accumulating into
a 32-row psum band; the two 512-col halves of a pair are emitted skewed
(s0 of pair idx, then s1 of pair idx-1) so consecutive matmuls hit
different PE column bands -- the PE streams up to 4 cols/cycle across 4
concurrent bands instead of serializing at 1 col/cycle.
Softmax: exp without row-max stabilizer (scores bounded); adj mask folded
into the post-transpose PSUM eviction (attT = pst * adjT).  The whole
i-tile tail (exp/transpose/mask/att-matmul/norm) is software-pipelined
into the NEXT i-tile's E2 stream so no engine drains at the boundary.
Final: out[i, :] = (att_unnorm @ [feat|1]) / rowsum.
"""
import sys

sys.path.insert(0, "/opt/trn_rl_repo")

from contextlib import ExitStack

import numpy as np

import concourse.bass as bass  # noqa: F401
import concourse.tile as tile
from concourse import bacc, mybir
from concourse.bass_utils import run_bass_kernel_spmd

B, T, C_IN, D = 2, 1024, 128, 64
N_CORES = 8
ROWS = (B * T) // N_CORES  # 256 destination rows per core
CPB = N_CORES // B  # cores per batch
NT = T // 128  # token tiles
NIT = ROWS // 128  # i-tiles per core
NPAIR = 64  # row pairs per i-tile
NSLOT = 16  # pair slots per 32-row psum band

FP32 = mybir.dt.float32
FP16 = mybir.dt.float16
OP = mybir.AluOpType
AF = mybir.ActivationFunctionType


def _on_scalar(idx):
    # 17 of 64 E2 tiles per i-tile on the ACT engine
    return idx % 4 == 2 or idx == 33


def _emit(ctx, tc, nc, hot_in, cold_in, adjT, out):
    singles = ctx.enter_context(tc.tile_pool(name="singles", bufs=1))
    hot = singles.tile([128, 2048], FP16)  # power-of-two row pitch
    cold = singles.tile([128, 128 + NT * (C_IN + 1)], FP16)
    dum = singles.tile([128, 2], FP16)
    qT2 = hot[:, 0:T]
    kpair16 = hot[:, T : T + ROWS // 2]
    A32s = hot[:, T + ROWS // 2 :]
    kpair = singles.tile([128, ROWS // 2], FP32)
    ident16 = cold[:, 0:128]
    feat16 = cold[:, 128:]

    # DMA triggers cost ~650ns serial per issuing sequencer: one hot DMA
    # first on sync, cold/adjT on the otherwise idle gpsimd sequencer.
    # one dma_start = one DMA queue ring: split the hot tile across three
    # triggers on three sequencers (parallel trigger + ring parallelism)
    nc.sync.dma_start(hot[:, 0:512], hot_in[:, 0:512])
    nc.scalar.dma_start(hot[:, 512:T], hot_in[:, 512:T])
    nc.sync.dma_start(hot[:, T:2048], hot_in[:, T:2048])
    adj_sb = []
    adjpool = ctx.enter_context(tc.tile_pool(name="adjp", bufs=NIT))
    for it in range(NIT):
        at = adjpool.tile([128, T], FP16, tag="adjT", name=f"adjT_{it}")
        adj_sb.append(at)

    # trigger the ACT table load (Exp) immediately, no data deps
    nc.scalar.memzero(dum[:])
    nc.scalar.activation(dum[:], dum[:], AF.Exp)
    # tensor_scalar needs an fp32 scalar: upconvert kpair once on ACT
    nc.scalar.copy(kpair[:], kpair16)

    e2pool = ctx.enter_context(tc.tile_pool(name="e2", bufs=12))
    softpool = ctx.enter_context(tc.tile_pool(name="soft", bufs=2))
    smallpool = ctx.enter_context(tc.tile_pool(name="small", bufs=2))
    attTpool = ctx.enter_context(tc.tile_pool(name="attT", bufs=2))
    outpool = ctx.enter_context(tc.tile_pool(name="outp", bufs=2))
    ps_scores = ctx.enter_context(tc.tile_pool(name="ps_s", bufs=2, space="PSUM"))
    ps_tr = ctx.enter_context(tc.tile_pool(name="ps_tr", bufs=2, space="PSUM"))
    ps_out = ctx.enter_context(tc.tile_pool(name="ps_o", bufs=2, space="PSUM"))

    W = C_IN + 1
    state = {}  # tail state of the previous i-tile

    def tail_exp(st, lo, step):
        nc.scalar.activation(
            st["pexp"][:, lo : lo + step], st["s"][:, lo : lo + step], AF.Exp
        )

    def tail_transpose(st, t):
        nc.tensor.transpose(
            st["pst"][:, t * 128 : (t + 1) * 128],
            st["pexp"][:, t * 128 : (t + 1) * 128],
            ident16,
        )

    def tail_mask(st, lo, step):
        nc.vector.tensor_tensor(
            st["attT"][:, lo : lo + step],
            st["pst"][:, lo : lo + step],
            adj_sb[st["it"]][:, lo : lo + step],
            OP.mult,
        )

    def tail_attmm(st, t):
        nc.tensor.matmul(
            st["po"][:],
            st["attT"][:, t * 128 : (t + 1) * 128],
            feat16[:, t * W : (t + 1) * W],
            start=(t == 0),
            stop=(t == NT - 1),
            skip_group_check=True,
        )

    def tail_norm_and_out(st):
        inv = smallpool.tile([128, 1], FP32, tag="inv")
        nc.vector.reciprocal(inv[:], st["po"][:, C_IN : C_IN + 1])
        out_sb = outpool.tile([128, C_IN], FP32, tag="out")
        nc.scalar.mul(out_sb[:], st["po"][:, 0:C_IN], inv[:])
        it = st["it"]
        nc.gpsimd.dma_start(out[it * 128 : (it + 1) * 128, :], out_sb[:])

    for it in range(NIT):
        s = ps_scores.tile([128, T], FP32, tag="s")  # 2 banks; halves by matmul
        prev = state if it > 0 else None
        pend = None  # (e2, lhsT, g, first, last) awaiting its 512:T half
        for idx in range(NPAIR):
            q, g = divmod(idx, 4)
            p = NSLOT * g + q
            P = it * NPAIR + p
            e2 = e2pool.tile([128, T], FP16, tag="e2")
            kcol = kpair[:, P : P + 1]
            if _on_scalar(idx):
                nc.scalar.activation(e2[:], qT2, AF.Relu, bias=kcol)
            else:
                nc.vector.tensor_scalar(e2[:], qT2, kcol, 0.0, OP.add, OP.max)
            lhsT = A32s[:, 32 * q : 32 * q + 32]
            first, last = q == 0, q == NSLOT - 1
            nc.tensor.matmul(
                s[32 * g : 32 * g + 32, 0:512],
                lhsT,
                e2[:, 0:512],
                start=first,
                stop=last,
                tile_position=(0, 32 * g),
                skip_group_check=True,
            )
            if pend is not None:
                pe2, plhsT, pg, pfirst, plast = pend
                nc.tensor.matmul(
                    s[32 * pg : 32 * pg + 32, 512:T],
                    plhsT,
                    pe2[:, 512:T],
                    start=pfirst,
                    stop=plast,
                    tile_position=(0, 32 * pg),
                    skip_group_check=True,
                )
            pend = (e2, lhsT, g, first, last)

            # cold inputs (identity/feat blocks/adj): trigger their DMAs
            # after the E2 stream is underway -- data needed from i-tile 1 on
            if it == 0:
                if idx == 8:
                    nc.gpsimd.dma_start(cold[:], cold_in[:, :])
                elif idx == 12:
                    nc.gpsimd.dma_start(adj_sb[0][:], adjT[:, 0:T])
                elif idx == 16:
                    nc.gpsimd.dma_start(adj_sb[1][:], adjT[:, T : 2 * T])

            # software-pipelined tail of the previous i-tile
            if prev is not None:
                if idx == 4:
                    tail_exp(prev, 0, T)
                elif 6 <= idx <= 13:
                    tail_transpose(prev, idx - 6)
                elif idx == 15:
                    tail_mask(prev, 0, T)
                elif 17 <= idx <= 24:
                    tail_attmm(prev, idx - 17)
                elif idx == 26:
                    tail_norm_and_out(prev)

        pe2, plhsT, pg, pfirst, plast = pend
        nc.tensor.matmul(
            s[32 * pg : 32 * pg + 32, 512:T],
            plhsT,
            pe2[:, 512:T],
            start=pfirst,
            stop=plast,
            tile_position=(0, 32 * pg),
            skip_group_check=True,
        )

        state = {
            "it": it,
            "s": s,
            "pexp": softpool.tile([128, T], FP16, tag="pexp", name=f"pexp_{it}"),
            "pst": ps_tr.tile([128, T], FP16, tag="tr", name=f"pst_{it}"),
            "attT": attTpool.tile([128, T], FP16, tag="attT", name=f"attT_{it}"),
            "po": ps_out.tile([128, W], FP32, tag="o", name=f"po_{it}"),
        }

    # final i-tile's tail, pipelined in j-quarters to shorten the exposed chain
    st = state
    for hh in range(4):
        lo = hh * 256
        tail_exp(st, lo, 256)
        for t in range(lo // 128, (lo + 256) // 128):
            tail_transpose(st, t)
        tail_mask(st, lo, 256)
        for t in range(lo // 128, (lo + 256) // 128):
            tail_attmm(st, t)
    tail_norm_and_out(st)


_PROGRAM = None


def build_program():
    global _PROGRAM
    if _PROGRAM is not None:
        return _PROGRAM
    nc = bacc.Bacc("TRN2", target_bir_lowering=False, debug=False, num_devices=N_CORES)
    hot_in = nc.dram_tensor("hot", [128, 2048], FP16, kind="ExternalInput")
    cold_in = nc.dram_tensor("cold", [128, 128 + NT * (C_IN + 1)], FP16, kind="ExternalInput")
    adjT = nc.dram_tensor("adjT", [128, NIT * T], FP16, kind="ExternalInput")
    out = nc.dram_tensor("out", [ROWS, C_IN], FP32, kind="ExternalOutput")
    with tile.TileContext(nc) as tc:
        with ExitStack() as ctx:
            _emit(ctx, tc, nc, hot_in, cold_in, adjT, out)
    nc.compile()
    _PROGRAM = nc
    return nc


def make_a32(a):
    a32 = np.zeros((128, NSLOT * 32), dtype=np.float16)
    for q in range(NSLOT):
        a32[0:64, 32 * q + 2 * q] = a
        a32[64:128, 32 * q + 2 * q + 1] = a
    return a32


def make_in_maps(feat, adj, W1, W2, a):
    feat = np.ascontiguousarray(feat, dtype=np.float32)
    adj = np.ascontiguousarray(adj, dtype=np.float32)
    W1 = np.asarray(W1, dtype=np.float32)
    W2 = np.asarray(W2, dtype=np.float32)
    a32 = make_a32(np.asarray(a, dtype=np.float32))
    ident = np.eye(128, dtype=np.float16)
    in_maps = []
    for b in range(B):
        feat16 = feat[b].astype(np.float16)  # [T, C_IN]
        q = (feat16.astype(np.float32) @ W1.T).astype(np.float16)  # [T, D]
        k = feat16.astype(np.float32) @ W2.T  # [T, D] fp32
        qT = np.ascontiguousarray(q.T)  # [D, T]
        qT2 = np.ascontiguousarray(np.concatenate([qT, qT], axis=0))  # [128, T]
        fb = feat16.reshape(NT, 128, C_IN).transpose(1, 0, 2)  # [128, NT, C_IN]
        fblk = np.concatenate(
            [fb, np.ones((128, NT, 1), dtype=np.float16)], axis=2
        ).reshape(128, NT * (C_IN + 1))
        fblk = np.ascontiguousarray(fblk)
        for cc in range(CPB):
            r0 = cc * ROWS
            krows = k[r0 : r0 + ROWS].astype(np.float32)  # [256, 64]
            kp = krows.reshape(ROWS // 2, 2, D)  # [p, two, d]
            kpair = np.ascontiguousarray(
                kp.transpose(1, 2, 0).reshape(128, ROWS // 2).astype(np.float32)
            )  # [[k2p d]; [k2p+1 d]] stacked -> [128, 128]
            arows = adj[b, r0 : r0 + ROWS].astype(np.float16)  # [256, 1024]
            ats = []
            for it in range(NIT):
                chunk = arows[it * 128 : (it + 1) * 128]  # [128 i, 1024 j]
                x = chunk.reshape(128, NT, 128)  # [ii, t, p]
                ats.append(x.transpose(2, 1, 0).reshape(128, T))  # [p, (t, ii)]
            adjT = np.ascontiguousarray(np.concatenate(ats, axis=1))
            hot = np.zeros((128, 2048), dtype=np.float16)
            hot[:, 0:T] = qT2
            hot[:, T : T + ROWS // 2] = kpair.astype(np.float16)
            hot[:, T + ROWS // 2 : T + ROWS // 2 + NSLOT * 32] = a32
            cold = np.ascontiguousarray(np.concatenate([ident, fblk], axis=1))
            in_maps.append({"hot": hot, "cold": cold, "adjT": adjT})
    return in_maps


def run(feat, adj, W1, W2, a, trace=False):
    nc = build_program()
    in_maps = make_in_maps(feat, adj, W1, W2, a)
    last_err = None
    for attempt in range(3):
        try:
            res = run_bass_kernel_spmd(
                nc, in_maps, core_ids=list(range(N_CORES)), trace=trace
            )
            outs = [np.asarray(res.results[c]["out"]) for c in range(N_CORES)]
            break
        except Exception as e:  # transient NRT device errors recover on retry
            last_err = e
            import time

            time.sleep(5)
    else:
        raise last_err
    full = np.concatenate(outs, axis=0).reshape(B, T, C_IN).astype(np.float32)
    return full, res


def kernel(feat, adj, W1, W2, a):
    full, _ = run(feat, adj, W1, W2, a)
    return full


# revision 13
# speedup vs baseline: 1.1641x; 1.0004x over previous
"""GATv2 attention layer (B=2, T=1024, C_IN=128, D=64) on 8 trn2 NeuronCores.

Sharding: flatten (B, T) destination rows -> 2048 rows, 256 per core.
Host packs inputs per core: qT2 = [q^T; q^T] fp16 (q = feat@W1^T), kpair
fp32 columns [k[2p]; k[2p+1]] (k = feat@W2^T), feat in 128-row blocks with
a ones column (rowsum trick), transposed adj tiles, the a-weight slot
matrix A32s, and an identity for PE transposes.

Per-core algorithm (i = destination row, j = source node, d = head dim 64):
  scores[i, j] = sum_d a[d] * relu(q[j, d] + k[i, d])
E2 tile per row-pair: relu(qT2 + kpair[:, p]) on DVE (396ns, 4x mode) or
ACT (1061ns) -- split ~47/17 per 64-pair i-tile to balance both engines.
Scores via PE matmuls with lhsT = A32s slot q, 16 pairs accumulating into
a 32-row psum band; the two 512-col halves of a pair are emitted skewed
(s0 of pair idx, then s1 of pair idx-1) so consecutive matmuls hit
different PE column bands -- the PE streams up to 4 cols/cycle across 4
concurrent bands instead of serializing at 1 col/cycle.
Softmax: exp without row-max stabilizer (scores bounded); adj mask folded
into the post-transpose PSUM eviction (attT = pst * adjT).  The whole
i-tile tail (exp/transpose/mask/att-matmul/norm) is software-pipelined
into the NEXT i-tile's E2 stream so no engine drains at the boundary.
Final: out[i, :] = (att_unnorm @ [feat|1]) / rowsum.
"""
import sys

sys.path.insert(0, "/opt/trn_rl_repo")

from contextlib import ExitStack

import numpy as np

import concourse.bass as bass  # noqa: F401
import concourse.tile as tile
from concourse import bacc, mybir
from concourse.bass_utils import run_bass_kernel_spmd

B, T, C_IN, D = 2, 1024, 128, 64
N_CORES = 8
ROWS = (B * T) // N_CORES  # 256 destination rows per core
CPB = N_CORES // B  # cores per batch
NT = T // 128  # token tiles
NIT = ROWS // 128  # i-tiles per core
NPAIR = 64  # row pairs per i-tile
NSLOT = 16  # pair slots per 32-row psum band

FP32 = mybir.dt.float32
FP16 = mybir.dt.float16
OP = mybir.AluOpType
AF = mybir.ActivationFunctionType


def _on_scalar(idx):
    # 17 of 64 E2 tiles per i-tile on the ACT engine
    return idx % 4 == 2 or idx == 33


def _emit(ctx, tc, nc, hot_in, cold_in, adjT, out):
    singles = ctx.enter_context(tc.tile_pool(name="singles", bufs=1))
    hot = singles.tile([128, 2048], FP16)  # power-of-two row pitch
    cold = singles.tile([128, 128 + NT * (C_IN + 1)], FP16)
    dum = singles.tile([128, 2], FP16)
    qT2 = hot[:, 0:T]
    kpair16 = hot[:, T : T + ROWS // 2]
    A32s = hot[:, T + ROWS // 2 :]
    kpair = singles.tile([128, ROWS // 2], FP32)
    ident16 = cold[:, 0:128]
    feat16 = cold[:, 128:]

    # DMA triggers cost ~650ns serial per issuing sequencer: one hot DMA
    # first on sync, cold/adjT on the otherwise idle gpsimd sequencer.
    # one dma_start = one DMA queue ring: split the hot tile across three
    # triggers on three sequencers (parallel trigger + ring parallelism)
    nc.sync.dma_start(hot[:, 0:512], hot_in[:, 0:512])
    nc.scalar.dma_start(hot[:, 512:T], hot_in[:, 512:T])
    nc.sync.dma_start(hot[:, T:2048], hot_in[:, T:2048])
    adj_sb = []
    adjpool = ctx.enter_context(tc.tile_pool(name="adjp", bufs=NIT))
    for it in range(NIT):
        at = adjpool.tile([128, T], FP16, tag="adjT", name=f"adjT_{it}")
        adj_sb.append(at)
    # cold data (identity/feat blocks/adj) is needed only from i-tile 0's
    # tail on; schedule its DMAs late so the early consumers' semaphore
    # thresholds exclude them
    with tc.tile_wait_until(0.012):
        nc.gpsimd.dma_start(cold[:], cold_in[:, :])
        nc.gpsimd.dma_start(adj_sb[0][:], adjT[:, 0:T])
        nc.gpsimd.dma_start(adj_sb[1][:], adjT[:, T : 2 * T])

    # trigger the ACT table load (Exp) immediately, no data deps
    nc.scalar.memzero(dum[:])
    nc.scalar.activation(dum[:], dum[:], AF.Exp)
    # tensor_scalar needs an fp32 scalar: upconvert kpair once on ACT
    nc.scalar.copy(kpair[:], kpair16)

    e2pool = ctx.enter_context(tc.tile_pool(name="e2", bufs=12))
    softpool = ctx.enter_context(tc.tile_pool(name="soft", bufs=2))
    smallpool = ctx.enter_context(tc.tile_pool(name="small", bufs=2))
    attTpool = ctx.enter_context(tc.tile_pool(name="attT", bufs=2))
    outpool = ctx.enter_context(tc.tile_pool(name="outp", bufs=2))
    ps_scores = ctx.enter_context(tc.tile_pool(name="ps_s", bufs=2, space="PSUM"))
    ps_tr = ctx.enter_context(tc.tile_pool(name="ps_tr", bufs=2, space="PSUM"))
    ps_out = ctx.enter_context(tc.tile_pool(name="ps_o", bufs=2, space="PSUM"))

    W = C_IN + 1
    state = {}  # tail state of the previous i-tile

    def tail_exp(st, lo, step):
        nc.scalar.activation(
            st["pexp"][:, lo : lo + step], st["s"][:, lo : lo + step], AF.Exp
        )

    def tail_transpose(st, t):
        nc.tensor.transpose(
            st["pst"][:, t * 128 : (t + 1) * 128],
            st["pexp"][:, t * 128 : (t + 1) * 128],
            ident16,
        )

    def tail_mask(st, lo, step):
        nc.vector.tensor_tensor(
            st["attT"][:, lo : lo + step],
            st["pst"][:, lo : lo + step],
            adj_sb[st["it"]][:, lo : lo + step],
            OP.mult,
        )

    def tail_attmm(st, t):
        nc.tensor.matmul(
            st["po"][:],
            st["attT"][:, t * 128 : (t + 1) * 128],
            feat16[:, t * W : (t + 1) * W],
            start=(t == 0),
            stop=(t == NT - 1),
            skip_group_check=True,
        )

    def tail_norm_and_out(st):
        inv = smallpool.tile([128, 1], FP32, tag="inv")
        nc.vector.reciprocal(inv[:], st["po"][:, C_IN : C_IN + 1])
        out_sb = outpool.tile([128, C_IN], FP32, tag="out")
        nc.scalar.mul(out_sb[:], st["po"][:, 0:C_IN], inv[:])
        it = st["it"]
        nc.gpsimd.dma_start(out[it * 128 : (it + 1) * 128, :], out_sb[:])

    for it in range(NIT):
        s = ps_scores.tile([128, T], FP32, tag="s")  # 2 banks; halves by matmul
        prev = state if it > 0 else None
        pend = None  # (e2, lhsT, g, first, last) awaiting its 512:T half
        for idx in range(NPAIR):
            q, g = divmod(idx, 4)
            p = NSLOT * g + q
            P = it * NPAIR + p
            e2 = e2pool.tile([128, T], FP16, tag="e2")
            kcol = kpair[:, P : P + 1]
            if _on_scalar(idx):
                nc.scalar.activation(e2[:], qT2, AF.Relu, bias=kcol)
            else:
                nc.vector.tensor_scalar(e2[:], qT2, kcol, 0.0, OP.add, OP.max)
            lhsT = A32s[:, 32 * q : 32 * q + 32]
            first, last = q == 0, q == NSLOT - 1
            nc.tensor.matmul(
                s[32 * g : 32 * g + 32, 0:512],
                lhsT,
                e2[:, 0:512],
                start=first,
                stop=last,
                tile_position=(0, 32 * g),
                skip_group_check=True,
            )
            if pend is not None:
                pe2, plhsT, pg, pfirst, plast = pend
                nc.tensor.matmul(
                    s[32 * pg : 32 * pg + 32, 512:T],
                    plhsT,
                    pe2[:, 512:T],
                    start=pfirst,
                    stop=plast,
                    tile_position=(0, 32 * pg),
                    skip_group_check=True,
                )
            pend = (e2, lhsT, g, first, last)

            # software-pipelined tail of the previous i-tile
            if prev is not None:
                if idx == 4:
                    tail_exp(prev, 0, T)
                elif 6 <= idx <= 13:
                    tail_transpose(prev, idx - 6)
                elif idx == 15:
                    tail_mask(prev, 0, T)
                elif 17 <= idx <= 24:
                    tail_attmm(prev, idx - 17)
                elif idx == 26:
                    tail_norm_and_out(prev)

        pe2, plhsT, pg, pfirst, plast = pend
        nc.tensor.matmul(
            s[32 * pg : 32 * pg + 32, 512:T],
            plhsT,
            pe2[:, 512:T],
            start=pfirst,
            stop=plast,
            tile_position=(0, 32 * pg),
            skip_group_check=True,
        )

        state = {
            "it": it,
            "s": s,
            "pexp": softpool.tile([128, T], FP16, tag="pexp", name=f"pexp_{it}"),
            "pst": ps_tr.tile([128, T], FP16, tag="tr", name=f"pst_{it}"),
            "attT": attTpool.tile([128, T], FP16, tag="attT", name=f"attT_{it}"),
            "po": ps_out.tile([128, W], FP32, tag="o", name=f"po_{it}"),
        }

    # final i-tile's tail, pipelined in j-quarters to shorten the exposed chain
    st = state
    for hh in range(4):
        lo = hh * 256
        tail_exp(st, lo, 256)
        for t in range(lo // 128, (lo + 256) // 128):
            tail_transpose(st, t)
        tail_mask(st, lo, 256)
        for t in range(lo // 128, (lo + 256) // 128):
            tail_attmm(st, t)
    tail_norm_and_out(st)


_PROGRAM = None


def build_program():
    global _PROGRAM
    if _PROGRAM is not None:
        return _PROGRAM
    nc = bacc.Bacc("TRN2", target_bir_lowering=False, debug=False, num_devices=N_CORES)
    hot_in = nc.dram_tensor("hot", [128, 2048], FP16, kind="ExternalInput")
    cold_in = nc.dram_tensor("cold", [128, 128 + NT * (C_IN + 1)], FP16, kind="ExternalInput")
    adjT = nc.dram_tensor("adjT", [128, NIT * T], FP16, kind="ExternalInput")
    out = nc.dram_tensor("out", [ROWS, C_IN], FP32, kind="ExternalOutput")
    with tile.TileContext(nc) as tc:
        with ExitStack() as ctx:
            _emit(ctx, tc, nc, hot_in, cold_in, adjT, out)
    nc.compile()
    _PROGRAM = nc
    return nc


def make_a32(a):
    a32 = np.zeros((128, NSLOT * 32), dtype=np.float16)
    for q in range(NSLOT):
        a32[0:64, 32 * q + 2 * q] = a
        a32[64:128, 32 * q + 2 * q + 1] = a
    return a32


def make_in_maps(feat, adj, W1, W2, a):
    feat = np.ascontiguousarray(feat, dtype=np.float32)
    adj = np.ascontiguousarray(adj, dtype=np.float32)
    W1 = np.asarray(W1, dtype=np.float32)
    W2 = np.asarray(W2, dtype=np.float32)
    a32 = make_a32(np.asarray(a, dtype=np.float32))
    ident = np.eye(128, dtype=np.float16)
    in_maps = []
    for b in range(B):
        feat16 = feat[b].astype(np.float16)  # [T, C_IN]
        q = (feat16.astype(np.float32) @ W1.T).astype(np.float16)  # [T, D]
        k = feat16.astype(np.float32) @ W2.T  # [T, D] fp32
        qT = np.ascontiguousarray(q.T)  # [D, T]
        qT2 = np.ascontiguousarray(np.concatenate([qT, qT], axis=0))  # [128, T]
        fb = feat16.reshape(NT, 128, C_IN).transpose(1, 0, 2)  # [128, NT, C_IN]
        fblk = np.concatenate(
            [fb, np.ones((128, NT, 1), dtype=np.float16)], axis=2
        ).reshape(128, NT * (C_IN + 1))
        fblk = np.ascontiguousarray(fblk)
        for cc in range(CPB):
            r0 = cc * ROWS
            krows = k[r0 : r0 + ROWS].astype(np.float32)  # [256, 64]
            kp = krows.reshape(ROWS // 2, 2, D)  # [p, two, d]
            kpair = np.ascontiguousarray(
                kp.transpose(1, 2, 0).reshape(128, ROWS // 2).astype(np.float32)
            )  # [[k2p d]; [k2p+1 d]] stacked -> [128, 128]
            arows = adj[b, r0 : r0 + ROWS].astype(np.float16)  # [256, 1024]
            ats = []
            for it in range(NIT):
                chunk = arows[it * 128 : (it + 1) * 128]  # [128 i, 1024 j]
                x = chunk.reshape(128, NT, 128)  # [ii, t, p]
                ats.append(x.transpose(2, 1, 0).reshape(128, T))  # [p, (t, ii)]
            adjT = np.ascontiguousarray(np.concatenate(ats, axis=1))
            hot = np.zeros((128, 2048), dtype=np.float16)
            hot[:, 0:T] = qT2
            hot[:, T : T + ROWS // 2] = kpair.astype(np.float16)
            hot[:, T + ROWS // 2 : T + ROWS // 2 + NSLOT * 32] = a32
            cold = np.ascontiguousarray(np.concatenate([ident, fblk], axis=1))
            in_maps.append({"hot": hot, "cold": cold, "adjT": adjT})
    return in_maps


def run(feat, adj, W1, W2, a, trace=False):
    nc = build_program()
    in_maps = make_in_maps(feat, adj, W1, W2, a)
    last_err = None
    for attempt in range(3):
        try:
            res = run_bass_kernel_spmd(
                nc, in_maps, core_ids=list(range(N_CORES)), trace=trace
            )
            outs = [np.asarray(res.results[c]["out"]) for c in range(N_CORES)]
            break
        except Exception as e:  # transient NRT device errors recover on retry
            last_err = e
            import time

            time.sleep(5)
    else:
        raise last_err
    full = np.concatenate(outs, axis=0).reshape(B, T, C_IN).astype(np.float32)
    return full, res


def kernel(feat, adj, W1, W2, a):
    full, _ = run(feat, adj, W1, W2, a)
    return full
